# revision 1
# baseline (speedup 1.0000x reference)
"""ConvLSTM1D v4: two H-blocks with private halos (super-step wavefront),
partition-aligned DVE ops (c-state lives on partitions 64:128), identity-MM
peephole adds, full-bank PSUM tiles, casting output DMAs.

Alignment rules (walrus birverifier): the two tensor INPUTS of a DVE op must
share their start partition; the OUTPUT may be at any partition offset.
PE matmuls move data across partitions (M index -> PSUM partition).

Data placement per block:
  c_t           -> cstg chunk tile, partitions 64:128 (bf16)
  h_t           -> cat chunk tile, partitions 64:128 (bf16; x in 0:64)
  wci/wcf/wco   -> const tiles at partitions 64:128
  IF = sigmoid([i;f])  [128] @0 ; tg @0 ; O, tct @64
"""

import numpy as np
import ml_dtypes

import concourse.bass as bass
import concourse.mybir as mybir
import concourse.tile as tile
from concourse import bacc

B, T, CIN, COUT, H = 8, 128, 64, 64, 256
NCORES = 8
BF16 = mybir.dt.bfloat16
F32 = mybir.dt.float32
AF = mybir.ActivationFunctionType
nbf16 = ml_dtypes.bfloat16

WE = 136
SS = 7
HALO = WE - 128
BLK = [(0, WE), (H - WE, H)]
OWN = [(0, 128, 0, 128), (HALO, WE, 128, H)]

USE_IDMM = True
SWAP = True
PSUM_BUFS = 2
NOM9 = False
HPOOL = False
PACK = False
ABEFORE = False
MIFEARLY = False
SYNCPOOL = False


def build_convlstm(T_steps=T, chunk=8):
    nc = bacc.Bacc(None, target_bir_lowering=False)

    xb = nc.declare_dram_parameter("xb", [T_steps, CIN, H], BF16, isOutput=False)
    h0b = nc.declare_dram_parameter("h0b", [COUT, H], BF16, isOutput=False)
    c0b = nc.declare_dram_parameter("c0b", [COUT, H], BF16, isOutput=False)
    wk = nc.declare_dram_parameter("wk", [6, 128, 128], BF16, isOutput=False)
    wcifb = nc.declare_dram_parameter("wcif", [128, H], BF16, isOutput=False)
    wcob = nc.declare_dram_parameter("wcob", [COUT, H], BF16, isOutput=False)
    biasb = nc.declare_dram_parameter("biasb", [256, 1], F32, isOutput=False)
    id128b = nc.declare_dram_parameter("id128", [128, 128], BF16, isOutput=False)
    hs = nc.declare_dram_parameter("hs", [T_steps, COUT, H], BF16, isOutput=True)
    cs = nc.declare_dram_parameter("cs", [T_steps, COUT, H], BF16, isOutput=True)

    nchunk = T_steps // chunk
    assert T_steps % chunk == 0

    with tile.TileContext(nc) as tc:
        with (
            tc.tile_pool(name="const", bufs=1) as const,
            tc.tile_pool(name="work", bufs=4) as work,
            tc.tile_pool(name="cstage", bufs=2) as cstage,
            tc.tile_pool(name="psum", bufs=2, space="PSUM") as psum,
        ):
            w_sb = const.tile([128, 6 * 128], BF16)
            nc.sync.dma_start(
                out=w_sb[:].rearrange("p (s m) -> p s m", s=6),
                in_=wk[:].rearrange("s k m -> k s m"),
            )
            id_sb = const.tile([128, 128], BF16)
            nc.sync.dma_start(out=id_sb[:], in_=id128b[:])
            # peephole weights live on partitions 64:128
            wcif_sb = const.tile([128, 2 * H], BF16)  # [64:128, 0:H]=wci, [64:128, H:2H]=wcf
            nc.sync.dma_start(out=wcif_sb[COUT:128, 0:H], in_=wcifb[0:COUT, :])
            nc.sync.dma_start(out=wcif_sb[COUT:128, H : 2 * H], in_=wcifb[COUT:128, :])
            wco_sb = const.tile([128, H], BF16)
            nc.sync.dma_start(out=wco_sb[COUT:128, :], in_=wcob[:])
            bias_sb = const.tile([128, 3], F32)
            nc.sync.dma_start(out=bias_sb[:, 0:1], in_=biasb[0:128])
            nc.sync.dma_start(out=bias_sb[0:COUT, 1:2], in_=biasb[128:192])
            nc.sync.dma_start(out=bias_sb[COUT:128, 2:3], in_=biasb[192:256])
            b_if = bias_sb[:, 0:1]
            b_g = bias_sb[0:COUT, 1:2]
            b_o = bias_sb[COUT:128, 2:3]

            cats = [[], []]
            for blk in range(2):
                g0, g1 = BLK[blk]
                for ci in range(nchunk):
                    ct = const.tile([128, chunk * WE], BF16, name=f"cat{blk}_{ci}")
                    cats[blk].append(ct)
                    nc.sync.dma_start(
                        out=ct[0:CIN, :].rearrange("p (t h) -> p t h", h=WE),
                        in_=xb[ci * chunk : (ci + 1) * chunk, :, g0:g1].rearrange(
                            "t c h -> c t h"
                        ),
                    )
                nc.sync.dma_start(out=cats[blk][0][CIN:128, 0:WE], in_=h0b[:, g0:g1])
            # c0 at partitions 64:128
            cb0 = []
            for blk in range(2):
                g0, g1 = BLK[blk]
                t0 = const.tile([128, WE], BF16, name=f"cb0_{blk}")
                cb0.append(t0)
                nc.sync.dma_start(out=t0[COUT:128, :], in_=c0b[:, g0:g1])

            REPEAT = 1
            h_last = [None, None]
            for rep in range(REPEAT):
              c_prev = [cb0[0][COUT:128, :], cb0[1][COUT:128, :]]
              cstg = [None, None]
              mif_pre = [None, None]

              def emit_mif(blk):
                  g0, g1 = BLK[blk]
                  m_if = work.tile(
                      [128, WE], BF16, tag=f"mif{blk}", name=f"mif{blk}_e"
                  )
                  nc.vector.tensor_mul(
                      m_if[0:COUT], wcif_sb[COUT:128, g0:g1], c_prev[blk]
                  )
                  nc.vector.tensor_mul(
                      m_if[COUT:128], wcif_sb[COUT:128, H + g0 : H + g1],
                      c_prev[blk],
                  )
                  return m_if
              for t in range(T_steps):
                ci, s = t // chunk, t % chunk

                if t > 0 and t % SS == 0:
                    catA, catB = cats[0][ci], cats[1][ci]
                    eng = nc.gpsimd if SYNCPOOL else nc.vector
                    eng.tensor_copy(
                        catA[CIN:128, s * WE + (WE - HALO) : s * WE + WE],
                        catB[CIN:128, s * WE + HALO : s * WE + 2 * HALO],
                    )
                    eng.tensor_copy(
                        catB[CIN:128, s * WE : s * WE + HALO],
                        catA[CIN:128, s * WE + (H - WE) : s * WE + (H - WE) + HALO],
                    )
                    eng.tensor_copy(
                        c_prev[0][:, WE - HALO : WE], c_prev[1][:, HALO : 2 * HALO]
                    )
                    eng.tensor_copy(
                        c_prev[1][:, 0:HALO],
                        c_prev[0][:, H - WE : H - WE + HALO],
                    )

                for blk in range(2):
                    if s == 0:
                        cstg[blk] = cstage.tile(
                            [128, chunk * WE], BF16, tag=f"cstg{blk}",
                            name=f"cstg{blk}_{ci}",
                        )
                    cat = cats[blk][ci][:, s * WE : (s + 1) * WE]
                    g0, g1 = BLK[blk]
                    if PACK:
                        pbf = psum.tile(
                            [128, 512], F32, tag=f"pb_{blk}", name=f"pb_{blk}_{t}",
                            bufs=PSUM_BUFS,
                        )
                        p0f, p1f = pbf, pbf
                        p0, p1 = pbf[:, 0:WE], pbf[:, 256 : 256 + WE]
                    else:
                        p0f = psum.tile(
                            [128, 512], F32, tag=f"p0_{blk}", name=f"p0_{blk}_{t}"
                        )
                        p1f = psum.tile(
                            [128, 512], F32, tag=f"p1_{blk}", name=f"p1_{blk}_{t}"
                        )
                        p0, p1 = p0f[:, 0:WE], p1f[:, 0:WE]

                    # peephole i,f: inputs @64, outputs packed to m_if @0/[64:]
                    if MIFEARLY and mif_pre[blk] is not None:
                        m_if = mif_pre[blk]
                        mif_pre[blk] = None
                    else:
                        m_if = emit_mif(blk)
                    halves = [(1, p1), (0, p0)] if SWAP else [(0, p0), (1, p1)]
                    for half, P in halves:
                        w = [
                            w_sb[:, (half * 3 + k) * 128 : (half * 3 + k + 1) * 128]
                            for k in range(3)
                        ]
                        if half == 0 and PACK:
                            # M7 opens the p0 group (m_if uses c_{t-1}, ready
                            # early). With SWAP, p1's group already cleared the
                            # shared bank; start=False plain-writes the p0
                            # region and leaves p1's has_written bits set for
                            # the late M9 accumulate.
                            nc.tensor.matmul(
                                P[:, :], id_sb[:], m_if[:], start=not SWAP,
                                stop=False, skip_group_check=True,
                            )
                        nc.tensor.matmul(
                            P[:, :], w[1], cat[:, :], start=(half == 1 or not PACK),
                            stop=False,
                        )
                        nc.tensor.matmul(
                            P[:, 1:WE], w[0], cat[:, 0 : WE - 1],
                            start=False, stop=False,
                        )
                        nc.tensor.matmul(
                            P[:, 0 : WE - 1], w[2], cat[:, 1:WE],
                            start=False, stop=True,
                        )

                    IF = work.tile([128, WE], BF16, tag=f"IF{blk}")
                    if USE_IDMM:
                        if not PACK:
                            nc.tensor.matmul(
                                p0[:, :], id_sb[:], m_if[:], start=False, stop=True,
                                skip_group_check=True,
                            )
                        nc.scalar.activation(IF[:], p0[:], AF.Sigmoid, bias=b_if)
                    else:
                        t_if = work.tile([128, WE], BF16, tag=f"tif{blk}")
                        nc.vector.tensor_add(t_if[:], m_if[:], p0[:])
                        nc.scalar.activation(IF[:], t_if[:], AF.Sigmoid, bias=b_if)
                    tg = work.tile([COUT, WE], BF16, tag=f"tg{blk}")
                    nc.scalar.activation(tg[:], p1[0:COUT], AF.Tanh, bias=b_g)

                    pr_i = work.tile([COUT, WE], BF16, tag=f"pri{blk}")
                    nc.vector.tensor_mul(pr_i[:], IF[0:COUT], tg[:])
                    pr_f = work.tile([COUT, WE], BF16, tag=f"prf{blk}")
                    nc.vector.tensor_mul(pr_f[:], IF[COUT:128], c_prev[blk])
                    c_new = cstg[blk][COUT:128, s * WE : (s + 1) * WE]
                    nc.vector.tensor_add(c_new, pr_i[:], pr_f[:])

                    O = work.tile([128, WE], BF16, tag=f"O{blk}")
                    if NOM9:
                        m_o = work.tile([128, WE], BF16, tag=f"mo{blk}")
                        nc.vector.tensor_mul(
                            m_o[COUT:128, :], wco_sb[COUT:128, g0:g1], c_new
                        )
                        t_o = work.tile([128, WE], BF16, tag=f"to{blk}")
                        nc.vector.tensor_add(
                            t_o[COUT:128, :], m_o[COUT:128, :], p1[COUT:128]
                        )
                        nc.scalar.activation(
                            O[COUT:128], t_o[COUT:128], AF.Sigmoid, bias=b_o
                        )
                    elif USE_IDMM:
                        m_o = work.tile([COUT, WE], BF16, tag=f"mo{blk}")
                        nc.vector.tensor_mul(m_o[:], wco_sb[COUT:128, g0:g1], c_new)
                        nc.tensor.matmul(
                            p1[COUT:128, :], id_sb[0:COUT, 0:COUT], m_o[:],
                            start=False, stop=True, tile_position=(0, 64),
                            skip_group_check=True,
                        )
                        if ABEFORE:
                            tct = work.tile([128, WE], BF16, tag=f"tct{blk}")
                            nc.scalar.activation(tct[COUT:128], c_new, AF.Tanh)
                        nc.scalar.activation(
                            O[COUT:128], p1[COUT:128], AF.Sigmoid, bias=b_o
                        )
                    else:
                        t_o = work.tile([128, WE], BF16, tag=f"to{blk}")
                        nc.vector.tensor_add(t_o[COUT:128], m_o[:], p1[COUT:128])
                        # inputs: m_o@0 vs p1[64:]@64 -> misaligned; fallback uses
                        # m_o written @64 instead
                        nc.scalar.activation(
                            O[COUT:128], t_o[COUT:128], AF.Sigmoid, bias=b_o
                        )
                    if not ABEFORE:
                        tct = work.tile([128, WE], BF16, tag=f"tct{blk}")
                        nc.scalar.activation(tct[COUT:128], c_new, AF.Tanh)

                    if t + 1 < T_steps:
                        nco, ns = (t + 1) // chunk, (t + 1) % chunk
                        h_dst = cats[blk][nco][CIN:128, ns * WE : (ns + 1) * WE]
                    else:
                        h_last[blk] = const.tile(
                            [128, WE], BF16, name=f"h_last{blk}"
                        )
                        h_dst = h_last[blk][CIN:128, :]
                    if HPOOL:
                        nc.gpsimd.tensor_mul(h_dst, O[COUT:128], tct[COUT:128])
                    else:
                        nc.vector.tensor_mul(h_dst, O[COUT:128], tct[COUT:128])

                    c_prev[blk] = c_new
                    if MIFEARLY and t + 1 < T_steps:
                        mif_pre[blk] = emit_mif(blk)

                if s == chunk - 1:
                    for blk in range(2):
                        l0, l1, gg0, gg1 = OWN[blk]
                        src = cstg[blk][COUT:128, :].rearrange(
                            "p (t h) -> p t h", h=WE
                        )[:, :, l0:l1]
                        dst = cs[ci * chunk : (ci + 1) * chunk, :, gg0:gg1].rearrange(
                            "t c h -> c t h"
                        )
                        nc.sync.dma_start(out=dst, in_=src)

            for blk in range(2):
                l0, l1, gg0, gg1 = OWN[blk]
                for cj in range(nchunk):
                    s0 = 1 if cj == 0 else 0
                    g0t = cj * chunk + s0 - 1
                    n = chunk - s0
                    src = cats[blk][cj][CIN:128, s0 * WE : chunk * WE].rearrange(
                        "p (t h) -> p t h", h=WE
                    )[:, :, l0:l1]
                    nc.sync.dma_start(
                        out=hs[g0t : g0t + n, :, gg0:gg1].rearrange("t c h -> c t h"),
                        in_=src,
                    )
                nc.sync.dma_start(
                    out=hs[T_steps - 1 : T_steps, :, gg0:gg1].rearrange(
                        "t c h -> c t h"
                    ),
                    in_=h_last[blk][CIN:128, l0:l1].rearrange(
                        "p (t h) -> p t h", h=l1 - l0
                    ),
                )

    nc.compile()
    return nc


def prep_inputs(x, h0, c0, conv_w, conv_b, Wci, Wcf, Wco, T_steps=T):
    x = np.asarray(x, dtype=np.float32)
    h0 = np.asarray(h0, dtype=np.float32)
    c0 = np.asarray(c0, dtype=np.float32)
    conv_w = np.asarray(conv_w, dtype=np.float32)
    conv_b = np.asarray(conv_b, dtype=np.float32)
    Wci = np.asarray(Wci, dtype=np.float32)[0, :, :, 0]
    Wcf = np.asarray(Wcf, dtype=np.float32)[0, :, :, 0]
    Wco = np.asarray(Wco, dtype=np.float32)[0, :, :, 0]

    wk = np.empty((6, 128, 128), dtype=nbf16)
    for half in range(2):
        for k in range(3):
            wk[half * 3 + k] = (
                conv_w[half * 128 : (half + 1) * 128, :, k].T.astype(nbf16)
            )
    wcif = np.concatenate([Wci, Wcf], axis=0).astype(nbf16)
    wcob = Wco.astype(nbf16)
    biasb = conv_b.reshape(256, 1).astype(np.float32)
    id128 = np.eye(128, dtype=nbf16)

    in_maps = []
    for i in range(NCORES):
        in_maps.append(
            {
                "xb": x[i, :T_steps, :, :, 0].astype(nbf16),
                "h0b": h0[i, :, :, 0].astype(nbf16),
                "c0b": c0[i, :, :, 0].astype(nbf16),
                "wk": wk,
                "wcif": wcif,
                "wcob": wcob,
                "biasb": biasb,
                "id128": id128,
            }
        )
    return in_maps


_NC_CACHE = {}


def kernel(x, h0, c0, conv_w, conv_b, Wci, Wcf, Wco):
    from concourse.bass_utils import run_bass_kernel_spmd

    if T not in _NC_CACHE:
        _NC_CACHE[T] = build_convlstm(T)
    nc = _NC_CACHE[T]
    in_maps = prep_inputs(x, h0, c0, conv_w, conv_b, Wci, Wcf, Wco)
    res = run_bass_kernel_spmd(nc, in_maps, core_ids=list(range(NCORES)))
    outs = res.results
    hs = np.stack(
        [outs[i]["hs"].astype(np.float32) for i in range(NCORES)], axis=0
    )[..., None]
    cs = np.stack(
        [outs[i]["cs"].astype(np.float32) for i in range(NCORES)], axis=0
    )[..., None]
    return hs, cs

